# revision 73
# baseline (speedup 1.0000x reference)
"""Trainium2 Bass kernel for the DeepEquilibriumModel (Anderson-accelerated DEQ).

Problem: 12 unrolled iterations of
    f(z) = tanh(z @ W1 + x @ Wx + b1) @ W2 + b2
with Anderson mixing (M=5, beta=1, lam=1e-4) from iteration 5 on.

Sharding: data parallel over the 2048 = B*S rows; 8 cores get 256 rows each
(cores 0-3 hold batch 0, cores 4-7 batch 1). Weights replicated. The Anderson
normal equations need global row sums per batch element -> tiny per-group
AllReduce ([1,20] floats, groups {0..3} / {4..7}).

Everything on-chip is transposed ([feature, row]); bf16 for all big tensors
(validated: 0.4% noise on f per iteration -> ~5e-3 final rel err, gate 2e-2).

Key restructure vs a straightforward port (z-space GEMM1 each iter):
  u_j := W1.T @ f_j + (Wx.T @ x + b1)      (computed right after f_j, i.e.
                                            DURING the AllReduce+solve window)
  z_{i+1} = sum_k c_k f_{i-k}   (coeffs c sum to 1)
  =>  pre-tanh input v_{i+1} = sum_k c_k u_{i-k}    (scaled-identity matmuls
                                                     on the PE, not DVE passes)
so the PE never idles long enough for the HAM clock gate to re-throttle, and
the serial AllReduce/solve chain overlaps the u-GEMM. GEMM2 runs m-chunk-major
so g and the Gram dots pipeline chunk-wise behind it.
"""

import os as _os

import numpy as np
import ml_dtypes

from concourse import bacc, bass, mybir, tile
from concourse.bass_utils import run_bass_kernel_spmd

B, S, D, F = 2, 1024, 512, 2048
MAX_ITER, M, LAM = int(_os.environ.get("K_ITERS", "12")), 5, 1e-4
NCORES = 8
RPC = (B * S) // NCORES      # rows per core = 256
KD = D // 128                # 4 chunks over D
KF = F // 128                # 16 chunks over F
MD = D // 128                # 4 output chunks over D

FP = mybir.dt.float32
BF = mybir.dt.bfloat16
ALU = mybir.AluOpType
ACT = mybir.ActivationFunctionType

RGROUPS = [[0, 1, 2, 3], [4, 5, 6, 7]]

N_DUMMY = int(_os.environ.get("K_DUMMY", "28"))      # keep-HAM-warm matmuls
N_WARM = int(_os.environ.get("K_CC_WARMUP", "2"))    # collective path warmups
USE_AG = _os.environ.get("K_USE_AG", "1") == "1"     # AllGather + local sum
USE_A2A = _os.environ.get("K_USE_A2A", "1") == "1"   # mesh AllToAll as AllGather
NDOT = 20                                            # 5 dots x 4 m-chunk slots
NEDOT = 84                                           # batched early dots
EB = int(_os.environ.get("K_EBATCH", "4"))           # iters 0..EB in one batched AllReduce


def _emit(nc: bass.Bass):
    v = nc.vector
    sc = nc.scalar
    gp = nc.gpsimd

    xT_d = nc.dram_tensor("xT", [D, RPC], BF, kind="ExternalInput")
    W1_d = nc.dram_tensor("W1", [D, F], BF, kind="ExternalInput")
    Wx_d = nc.dram_tensor("Wx", [D, F], BF, kind="ExternalInput")
    W2_d = nc.dram_tensor("W2", [F, D], BF, kind="ExternalInput")
    b1_d = nc.dram_tensor("b1", [F], FP, kind="ExternalInput")
    b2_d = nc.dram_tensor("b2", [D], FP, kind="ExternalInput")
    gm_d = nc.dram_tensor("gmask", [1, NCORES], FP, kind="ExternalInput")
    zout_d = nc.dram_tensor("zT_out", [D, RPC], FP, kind="ExternalOutput")

    with tile.TileContext(nc) as tc:
        with (
            tc.tile_pool(name="const", bufs=1) as cp,
            tc.tile_pool(name="state", bufs=1) as sp,
            tc.tile_pool(name="psv", bufs=1, space="PSUM") as ppv,
            tc.tile_pool(name="psw", bufs=2, space="PSUM") as ppw,
            tc.tile_pool(name="psu", bufs=2, space="PSUM") as ppu,
            tc.tile_pool(name="ps2", bufs=2, space="PSUM") as pp2,
            tc.tile_pool(name="pssm", bufs=1, space="PSUM") as pps,
            tc.tile_pool(name="dram", bufs=2, space="DRAM") as dp,
        ):
            # ---------------- constants / weights ----------------
            W1p = cp.tile([128, KD * F], BF)          # (k,f) at [:, k*F + f*128]
            W2p = cp.tile([128, KF * D], BF)          # (f,m) at [:, f*D + m*128]
            Wxp = cp.tile([128, KD * F], BF)
            xTs = cp.tile([128, KD * RPC], BF)        # k at [:, k*RPC]
            xwxb = cp.tile([128, KF * RPC], BF)       # Wx.T x + b1, f at [:, f*RPC]
            b1t = cp.tile([128, KF], FP)
            b2t = cp.tile([128, MD], FP)
            ones_col = cp.tile([128, 1], FP)
            ones_row = cp.tile([1, 128], FP)
            onesq = cp.tile([128, 128], FP)
            identB = cp.tile([128, 128], BF)

            # load order = consumption order: x+Wx+b1 (xwx GEMM), W2 (GEMM2),
            # W1 (u-GEMM) last
            nc.sync.dma_start(b1t[:], b1_d.ap().rearrange("(f p) -> p f", p=128))
            nc.sync.dma_start(b2t[:], b2_d.ap().rearrange("(m p) -> p m", p=128))
            for k in range(KD):
                nc.sync.dma_start(xTs[:, k * RPC:(k + 1) * RPC], xT_d[k * 128:(k + 1) * 128, :])
            for k in range(KD):
                nc.sync.dma_start(Wxp[:, k * F:(k + 1) * F], Wx_d[k * 128:(k + 1) * 128, :])
            for f in range(KF):
                nc.sync.dma_start(W2p[:, f * D:(f + 1) * D], W2_d[f * 128:(f + 1) * 128, :])
            for k in range(KD):
                nc.sync.dma_start(W1p[:, k * F:(k + 1) * F], W1_d[k * 128:(k + 1) * 128, :])
            v.memset(ones_col[:], 1.0)
            v.memset(ones_row[:], 1.0)
            v.memset(onesq[:], 1.0)
            gp.affine_select(onesq[:], onesq[:], [[1, 128]], ALU.is_equal, 0.0,
                             base=0, channel_multiplier=-1)
            v.tensor_copy(identB[:], onesq[:])

            # ---------------- persistent state ----------------
            uh = [sp.tile([128, KF * RPC], BF, name=f"uh{j}") for j in range(M)]
            fh = [sp.tile([128, KD * RPC], BF, name=f"fh{j}") for j in range(M)]
            gh = [sp.tile([128, KD * RPC], BF, name=f"gh{j}") for j in range(M)]
            hT = sp.tile([128, KF * RPC], BF)
            zc = sp.tile([128, KD * RPC], BF)         # Anderson-combined z
            zstage = sp.tile([128, KD * RPC], FP)
            zt0 = sp.tile([128, KD * RPC], BF)
            zt1 = sp.tile([128, KD * RPC], BF)
            junk_dve = sp.tile([128, RPC], BF)
            junk_act = sp.tile([128, RPC], BF)
            dots_act = sp.tile([128, MD], FP)         # <g,g> per m-chunk
            dots_dve = sp.tile([128, 2 * MD], FP)     # <g,g_{i-j}> j=1,2 per chunk
            dots_gp = sp.tile([128, 2 * MD], FP)      # <g,g_{i-j}> j=3,4 per chunk
            # batched early (iters 0-4) dot accumulators: one AllReduce total
            edots_act = sp.tile([128, 5 * MD], FP)       # sq: iter i at [:, i*4+c]
            edots_dve = sp.tile([128, 4 * 4 * MD], FP)   # (i-1,j-1,c) i,j=1..4
            redp = sp.tile([1, NEDOT], FP)
            red2 = sp.tile([1, NDOT], FP)
            red2E = sp.tile([1, NEDOT], FP)
            red4g = sp.tile([1, 4 * NDOT], FP)
            red8g = sp.tile([1, NCORES * NDOT], FP)
            gmask = sp.tile([1, NCORES], FP)
            red8p = sp.tile([1, NCORES * NDOT], FP)
            red5 = sp.tile([1, M], FP)
            Pg = [sp.tile([1, 25], FP, name=f"pg{j}") for j in range(2)]
            cIb = sp.tile([128, M * 128], BF)         # scaled identities (bf16)
            aug = sp.tile([1, 20], FP)                # [HTH | HTy] augmented 4x5
            outer = sp.tile([1, 20], FP)
            t45 = sp.tile([1, 20], FP)
            rct = sp.tile([1, 4], FP)
            gam = sp.tile([1, 4], FP)
            csum = sp.tile([1, 1], FP)
            coeffs = sp.tile([1, M], FP)
            gst = sp.tile([1, 8], FP)

            nc.sync.dma_start(gmask[:], gm_d[:, :])
            v.memset(red8p[:], 0.0)
            v.memset(dots_act[:], 0.0)
            v.memset(dots_dve[:], 0.0)
            v.memset(dots_gp[:], 0.0)
            v.memset(edots_act[:], 0.0)
            v.memset(edots_dve[:], 0.0)
            v.memset(redp[:], 0.0)
            v.memset(Pg[0][:], 0.0)
            v.memset(Pg[1][:], 0.0)

            # warm the collective paths (first op of a kind pays big latency)
            wcc_in = dp.tile([1, NEDOT], FP, tag="ecci", name="wcci")
            wcc_out = dp.tile([1, NEDOT], FP, tag="ecco", name="wcco")
            gp.dma_start(wcc_in[:], redp[:])
            gp.collective_compute(
                "AllReduce", ALU.add, replica_groups=RGROUPS,
                ins=[wcc_in.opt()], outs=[wcc_out.opt()],
            )
            for w in range(N_WARM):
                if USE_A2A:
                    wag_in = dp.tile([1, NCORES * NDOT], FP, tag="cci", name="wagi")
                    wag_out = dp.tile([1, NCORES * NDOT], FP, tag="cco", name="wago")
                    gp.dma_start(wag_in[:], red8p[:])
                    gp.collective_compute(
                        "AllToAll", ALU.bypass,
                        replica_groups=[list(range(NCORES))],
                        ins=[wag_in.opt()], outs=[wag_out.opt()],
                    )
                elif USE_AG:
                    wag_in = dp.tile([1, NDOT], FP, tag="cci", name="wagi")
                    wag_out = dp.tile([1, 4 * NDOT], FP, tag="cco", name="wago")
                    gp.dma_start(wag_in[:], redp[:, 0:NDOT])
                    gp.collective_compute(
                        "AllGather", ALU.bypass, replica_groups=RGROUPS,
                        ins=[wag_in.opt()], outs=[wag_out.opt()],
                    )
                else:
                    wag_in = dp.tile([1, NDOT], FP, tag="cci", name="wagi")
                    wag_out = dp.tile([1, NDOT], FP, tag="cco", name="wago")
                    gp.dma_start(wag_in[:], redp[:, 0:NDOT])
                    gp.collective_compute(
                        "AllReduce", ALU.add, replica_groups=RGROUPS,
                        ins=[wag_in.opt()], outs=[wag_out.opt()],
                    )

            # ---------------- xwxb = Wx.T @ xT + b1 ; h0 = tanh(xwxb) ----------
            for f in range(KF):
                ps = ppv.tile([128, RPC], FP, tag="psv", name="psx")
                for k in range(KD):
                    nc.tensor.matmul(
                        ps[:],
                        Wxp[:, k * F + f * 128: k * F + (f + 1) * 128],
                        xTs[:, k * RPC:(k + 1) * RPC],
                        start=(k == 0), stop=(k == KD - 1),
                    )
                sc.activation(xwxb[:, f * RPC:(f + 1) * RPC], ps[:],
                              ACT.Identity, bias=b1t[:, f:f + 1], scale=1.0)
                sc.activation(hT[:, f * RPC:(f + 1) * RPC], ps[:],
                              ACT.Tanh, bias=b1t[:, f:f + 1], scale=1.0)

            # ---------------- main loop (fully unrolled) ----------------
            # z_ap: AP of z_i (for g); None at i=0 (z=0)
            z_ap = None

            for i in range(MAX_ITER):
                slot = i % M
                f_t, g_t = fh[slot], gh[slot]

                # ---- pre-tanh input + tanh -> hT (i>=1; i==0 done above) ----
                if 1 <= i <= M:
                    # z_i = f_{i-1}: v_i = u_{i-1} directly
                    up = uh[(i - 1) % M]
                    for f in range(KF):
                        sc.activation(hT[:, f * RPC:(f + 1) * RPC],
                                      up[:, f * RPC:(f + 1) * RPC], ACT.Tanh)
                elif i > M:
                    # v_i = sum_k c_k u_{i-1-k}: the stationary (scaled
                    # identity) is the same for every chunk, so adjacent
                    # f-chunks pair into one N=512 matmul (full PSUM bank),
                    # halving instruction count and tanh op count
                    for fp in range(KF // 2):
                        ps = ppw.tile([128, 2 * RPC], FP, tag="psw", name="psw")
                        for k in range(M):
                            nc.tensor.matmul(
                                ps[:],
                                cIb[:, k * 128:(k + 1) * 128],
                                uh[(i - 1 - k) % M][:, fp * 2 * RPC:(fp + 1) * 2 * RPC],
                                start=(k == 0), stop=(k == M - 1),
                            )
                        sc.activation(hT[:, fp * 2 * RPC:(fp + 1) * 2 * RPC],
                                      ps[:], ACT.Tanh)

                # ---- P history bookkeeping ----
                Pc, Pp = Pg[i % 2], Pg[(i + 1) % 2]
                P3c = Pc[:].rearrange("p (a b) -> p a b", a=5)
                P3p = Pp[:].rearrange("p (a b) -> p a b", a=5)
                if i == EB + 1:
                    # replay iters 0-3 from the single batched AllReduce, on
                    # the otherwise-idle GPSIMD engine (overlaps this
                    # iteration's GEMMs on PE / z ops on DVE)
                    def gp_fold4(dst, src):
                        # dst [1,n] = per-group-of-4 sums of src [1,4n]
                        n = dst.shape[-1]
                        s3 = src.rearrange("p (j c) -> p j c", j=n)
                        g3 = gst[:, 0:2 * n].rearrange("p (j c) -> p j c", j=n)
                        gp.tensor_tensor(g3, s3[:, :, 0:2], s3[:, :, 2:4],
                                         op=ALU.add)
                        gp.tensor_tensor(dst, g3[:, :, 0:1].rearrange(
                            "p j c -> p (j c)"), g3[:, :, 1:2].rearrange(
                            "p j c -> p (j c)"), op=ALU.add)

                    for ii in range(EB + 1):
                        Pcc, Ppp = Pg[ii % 2], Pg[(ii + 1) % 2]
                        P3cc = Pcc[:].rearrange("p (a b) -> p a b", a=5)
                        P3pp = Ppp[:].rearrange("p (a b) -> p a b", a=5)
                        if ii > 0:
                            gp.tensor_copy(P3cc[:, 1:5, 1:5], P3pp[:, 0:4, 0:4])
                        gp_fold4(red5[:, 0:1], red2E[:, ii * 4:(ii + 1) * 4])
                        if ii > 0:
                            base = NDOT + (ii - 1) * 16
                            gp_fold4(red5[:, 1:5], red2E[:, base:base + 16])
                        else:
                            gp.memset(red5[:, 1:5], 0.0)
                        gp.tensor_copy(Pcc[:, 0:5], red5[:, 0:5])
                        gp.tensor_copy(Pcc[:, 5:25:5], red5[:, 1:5])
                if i >= EB + 1:
                    # shift by one age (uses only last iter's P; runs early)
                    v.tensor_copy(P3c[:, 1:5, 1:5], P3p[:, 0:4, 0:4])
                if i >= M:
                    # precompute the old-pairs part of HTH (+ridge) so the
                    # post-AllReduce solve prep is shorter
                    T3 = t45[:].rearrange("p (a b) -> p a b", a=4)[:, :, 0:4]
                    v.tensor_copy(T3, P3c[:, 1:5, 1:5])
                    v.tensor_scalar(t45[:, 0:19:6], t45[:, 0:19:6], LAM, None,
                                    op0=ALU.add)

                # ---- GEMM2 m-chunk-major; g + dots pipeline behind it ----
                njd = min(i, M - 1)
                for m in range(MD):
                    ps2 = pp2.tile([128, RPC], FP, tag="ps2", name="ps2")
                    for f in range(KF):
                        nc.tensor.matmul(
                            ps2[:],
                            W2p[:, f * D + m * 128: f * D + (m + 1) * 128],
                            hT[:, f * RPC:(f + 1) * RPC],
                            start=(f == 0), stop=(f == KF - 1),
                        )
                    sl = slice(m * RPC, (m + 1) * RPC)
                    sc.activation(f_t[:, sl], ps2[:], ACT.Identity,
                                  bias=b2t[:, m:m + 1], scale=1.0)
                    # g straight from PSUM (doesn't wait on the bias ACT)
                    if i == 0:
                        v.tensor_scalar(g_t[:, sl], ps2[:], b2t[:, m:m + 1], None,
                                        op0=ALU.add)
                    else:
                        v.scalar_tensor_tensor(g_t[:, sl], ps2[:], b2t[:, m:m + 1],
                                               z_ap[:, sl],
                                               op0=ALU.add, op1=ALU.subtract)
                    da = (dots_act[:, m:m + 1] if i > EB
                          else edots_act[:, i * MD + m:i * MD + m + 1])
                    sc.activation(junk_act[:], g_t[:, sl], ACT.Square,
                                  accum_out=da)
                    for j in range(1, njd + 1):
                        if i > EB:
                            if j <= 2:
                                dd = dots_dve[:, (j - 1) * MD + m:(j - 1) * MD + m + 1]
                            else:
                                dd = dots_gp[:, (j - 3) * MD + m:(j - 3) * MD + m + 1]
                        else:
                            c0 = ((i - 1) * 4 + (j - 1)) * MD + m
                            dd = edots_dve[:, c0:c0 + 1]
                        v.scalar_tensor_tensor(
                            junk_dve[:], g_t[:, sl], 1.0,
                            gh[(i - j) % M][:, sl],
                            op0=ALU.bypass, op1=ALU.mult,
                            accum_out=dd,
                        )

                # ---- partition-reduce dots, ship through AllReduce ----
                pball = pps.tile([128, 96], FP, tag="psmall", name="pball")
                if i == EB:
                    # one batched AllReduce for the whole early history
                    nc.tensor.matmul(pball[0:1, 0:NDOT], ones_col[:],
                                     edots_act[:], start=True, stop=True)
                    nc.tensor.matmul(pball[0:1, NDOT:NEDOT], ones_col[:],
                                     edots_dve[:], start=True, stop=True)
                    sc.activation(redp[:], pball[0:1, 0:NEDOT], ACT.Copy)
                    ecc_in = dp.tile([1, NEDOT], FP, tag="ecci", name="ecci")
                    ecc_out = dp.tile([1, NEDOT], FP, tag="ecco", name="ecco")
                    nc.sync.dma_start(ecc_in[:], redp[:])
                    gp.collective_compute(
                        "AllReduce", ALU.add, replica_groups=RGROUPS,
                        ins=[ecc_in.opt()], outs=[ecc_out.opt()],
                    )
                    nc.sync.dma_start(red2E[:], ecc_out[:])
                elif i > EB:
                    nc.tensor.matmul(pball[0:1, 0:MD], ones_col[:], dots_act[:],
                                     start=True, stop=True)
                    nc.tensor.matmul(pball[0:1, MD:3 * MD], ones_col[:], dots_dve[:],
                                     start=True, stop=True)
                    nc.tensor.matmul(pball[0:1, 3 * MD:NDOT], ones_col[:], dots_gp[:],
                                     start=True, stop=True)
                    if USE_A2A:
                        # 8-rank mesh AllToAll as a masked AllGather: shard r
                        # carries my dots if rank r is in my batch group, else
                        # zeros -- the fold then sums all 8 shards and the
                        # other group contributes nothing
                        v.tensor_tensor(
                            red8p[:].rearrange("p (r f) -> p r f", r=NCORES),
                            pball[0:1, 0:NDOT].rearrange(
                                "p (r f) -> p r f", r=1).broadcast_to(
                                    [1, NCORES, NDOT]),
                            gmask[:].rearrange("p (r f) -> p r f", f=1
                                               ).broadcast_to([1, NCORES, NDOT]),
                            op=ALU.mult)
                        cc_in = dp.tile([1, NCORES * NDOT], FP, tag="cci", name="cci")
                        nc.sync.dma_start(cc_in[:], red8p[:])
                        cc_out = dp.tile([1, NCORES * NDOT], FP, tag="cco", name="cco")
                        gp.collective_compute(
                            "AllToAll", ALU.bypass,
                            replica_groups=[list(range(NCORES))],
                            ins=[cc_in.opt()], outs=[cc_out.opt()],
                        )
                        nc.sync.dma_start(red8g[:], cc_out[:])
                    elif USE_AG:
                        sc.activation(redp[:, 0:NDOT], pball[0:1, 0:NDOT], ACT.Copy)
                        cc_in = dp.tile([1, NDOT], FP, tag="cci", name="cci")
                        nc.sync.dma_start(cc_in[:], redp[:, 0:NDOT])
                        cc_out = dp.tile([1, 4 * NDOT], FP, tag="cco", name="cco")
                        gp.collective_compute(
                            "AllGather", ALU.bypass, replica_groups=RGROUPS,
                            ins=[cc_in.opt()], outs=[cc_out.opt()],
                        )
                        nc.sync.dma_start(red4g[:], cc_out[:])
                    else:
                        sc.activation(redp[:, 0:NDOT], pball[0:1, 0:NDOT], ACT.Copy)
                        cc_in = dp.tile([1, NDOT], FP, tag="cci", name="cci")
                        cc_out = dp.tile([1, NDOT], FP, tag="cco", name="cco")
                        nc.sync.dma_start(cc_in[:], redp[:, 0:NDOT])
                        gp.collective_compute(
                            "AllReduce", ALU.add, replica_groups=RGROUPS,
                            ins=[cc_in.opt()], outs=[cc_out.opt()],
                        )
                        nc.sync.dma_start(red2[:], cc_out[:])

                # ---- u_i = W1.T @ f_i + xwxb  (runs during the AllReduce).
                # For early iterations the DVE is idle and has no post-
                # collective chain queued, so the xwxb fold rides a DVE
                # epilogue there instead of 16 identity matmuls; Anderson
                # iterations keep the PE fold so the DVE queue stays clear
                # ahead of the solve.
                if i < MAX_ITER - 1:
                    u_t = uh[slot]
                    fold_on_pe = i > EB
                    for f in range(KF):
                        psu = ppu.tile([128, RPC], FP, tag="psu", name="psu")
                        if fold_on_pe:
                            nc.tensor.matmul(psu[:], identB[:],
                                             xwxb[:, f * RPC:(f + 1) * RPC],
                                             start=True, stop=False)
                        for k in range(KD):
                            nc.tensor.matmul(
                                psu[:],
                                W1p[:, k * F + f * 128: k * F + (f + 1) * 128],
                                f_t[:, k * RPC:(k + 1) * RPC],
                                start=(not fold_on_pe and k == 0),
                                stop=(k == KD - 1),
                            )
                        if fold_on_pe:
                            sc.activation(u_t[:, f * RPC:(f + 1) * RPC], psu[:],
                                          ACT.Copy)
                        else:
                            v.scalar_tensor_tensor(
                                u_t[:, f * RPC:(f + 1) * RPC], psu[:], 1.0,
                                xwxb[:, f * RPC:(f + 1) * RPC],
                                op0=ALU.bypass, op1=ALU.add)
                # keep the PE's HAM clock warm through the post-collective
                # solve window (first two share PSUM slots with the tail
                # u-copies -- freed within ~0.5us)
                if i >= M:
                    for dmy in range(N_DUMMY):
                        psd = ppu.tile([128, RPC], FP, tag="psu", name="psu")
                        nc.tensor.matmul(psd[:], identB[:], xwxb[:, 0:RPC],
                                         start=True, stop=True)

                if i <= EB:
                    z_ap = f_t
                    continue

                # ---- fold AllReduced/AllGathered dots, insert into P ----
                if USE_A2A:
                    v.tensor_reduce(
                        red5[:],
                        red8g[:].rearrange("p (r j c) -> p j r c", r=NCORES, j=M),
                        axis=mybir.AxisListType.XY, op=ALU.add)
                elif USE_AG:
                    # one strided-view reduce: [1,(r j c)] -> [1,j,(r c)] -> [1,j]
                    v.tensor_reduce(
                        red5[:],
                        red4g[:].rearrange("p (r j c) -> p j r c", r=4, j=M),
                        axis=mybir.AxisListType.XY, op=ALU.add)
                else:
                    v.tensor_reduce(red5[:],
                                    red2[:].rearrange("p (j c) -> p j c", j=M),
                                    axis=mybir.AxisListType.X, op=ALU.add)
                if i < M:
                    v.tensor_copy(Pc[:, 0:5], red5[:, 0:5])
                    v.tensor_copy(Pc[:, 5:25:5], red5[:, 1:5])
                    z_ap = f_t
                    continue

                # ---- finish [HTH + lam I | HTy] (4x5, in aug) ----
                # HTH[a][b] = P00 - P0b - Pa0 + Pab ; HTy[a] = P00 - Pa0.
                # P is symmetric so the new row/col 0 IS red5 -- read it
                # directly; the P inserts happen after the solve, off the
                # critical path (only next iteration's shift needs them).
                A3 = aug[:].rearrange("p (a b) -> p a b", a=4)
                H3 = A3[:, :, 0:4]
                T3 = t45[:].rearrange("p (a b) -> p a b", a=4)[:, :, 0:4]
                r5r = red5[:, 1:5].rearrange("p (a b) -> p a b", a=1
                                             ).broadcast_to([1, 4, 4])
                r5c = red5[:, 1:5].rearrange("p (a b) -> p a b", b=1
                                             ).broadcast_to([1, 4, 4])
                v.scalar_tensor_tensor(T3, r5r, -1.0, T3,
                                       op0=ALU.mult, op1=ALU.add)
                v.tensor_tensor(T3, T3, r5c, op=ALU.subtract)
                v.tensor_scalar(H3, T3, red5[:, 0:1], None, op0=ALU.add)
                v.scalar_tensor_tensor(A3[:, :, 4:5],
                                       red5[:, 1:5].rearrange(
                                           "p (a b) -> p a b", b=1),
                                       -1.0,
                                       red5[:, 0:1].broadcast_to([1, 4]).rearrange(
                                           "p (a b) -> p a b", b=1),
                                       op0=ALU.mult, op1=ALU.add)

                # ---- Gauss-Jordan on [A|y]: gamma lands in aug col 4 ----
                O3 = outer[:].rearrange("p (a b) -> p a b", a=4)
                for p in range(4):
                    rp = rct[:, p:p + 1]
                    v.reciprocal(rp, aug[:, p * 5 + p:p * 5 + p + 1])
                    v.scalar_tensor_tensor(
                        O3, A3[:, :, p:p + 1].broadcast_to([1, 4, 5]), rp,
                        A3[:, p:p + 1, :].broadcast_to([1, 4, 5]),
                        op0=ALU.mult, op1=ALU.mult)
                    v.tensor_tensor(A3, A3, O3, op=ALU.subtract)
                    v.tensor_scalar(aug[:, p * 5:(p + 1) * 5],
                                    outer[:, p * 5:(p + 1) * 5], rp, None,
                                    op0=ALU.mult)

                # ---- coeffs = [1 - sum(gamma), gamma] -> broadcast + cI ----
                v.tensor_copy(gam[:], aug[:, 4:20:5])
                v.tensor_reduce(csum[:], gam[:], axis=mybir.AxisListType.X, op=ALU.add)
                v.tensor_scalar(coeffs[:, 0:1], csum[:], -1.0, 1.0,
                                op0=ALU.mult, op1=ALU.add)
                v.tensor_copy(coeffs[:, 1:5], gam[:])
                psb = pball[:, 24:29]
                nc.tensor.matmul(psb, ones_row[:], coeffs[:], start=True, stop=True)
                for k in range(M):
                    v.tensor_scalar(cIb[:, k * 128:(k + 1) * 128],
                                    identB[:], psb[:, k:k + 1], None, op0=ALU.mult)
                # deferred P inserts (next iteration's shift needs them)
                v.tensor_copy(Pc[:, 0:5], red5[:, 0:5])
                v.tensor_copy(Pc[:, 5:25:5], red5[:, 1:5])

                # ---- z_{i+1} = sum_k c_k f_{i-k} on DVE (hidden under next
                #      iteration's PE phase); full-width ops to cut per-op
                #      overhead; final iteration uses PE instead ----
                if i < MAX_ITER - 1:
                    v.tensor_scalar(zt0[:], fh[i % M][:], psb[:, 0:1], None,
                                    op0=ALU.mult)
                    cur = zt0
                    for k in range(1, M):
                        dst = zc if k == M - 1 else (zt1 if cur is zt0 else zt0)
                        v.scalar_tensor_tensor(dst[:], fh[(i - k) % M][:],
                                               psb[:, k:k + 1], cur[:],
                                               op0=ALU.mult, op1=ALU.add)
                        cur = dst
                    z_ap = zc

            # ---------------- final z = sum_k c_k f_{11-k} on PE ----------------
            li = MAX_ITER - 1
            for m in range(MD):
                ps = ppv.tile([128, RPC], FP, tag="psv", name="psf")
                for k in range(M):
                    nc.tensor.matmul(
                        ps[:], cIb[:, k * 128:(k + 1) * 128],
                        fh[(li - k) % M][:, m * RPC:(m + 1) * RPC],
                        start=(k == 0), stop=(k == M - 1),
                    )
                sc.activation(zstage[:, m * RPC:(m + 1) * RPC], ps[:], ACT.Copy)
            for k in range(KD):
                nc.sync.dma_start(zout_d[k * 128:(k + 1) * 128, :],
                                  zstage[:, k * RPC:(k + 1) * RPC])

    nc.compile()
    nc.finalize()
    return nc


_NC = None


def _get_nc():
    global _NC
    if _NC is None:
        nc = bacc.Bacc(trn_type="TRN2", debug=False, num_devices=NCORES)
        _NC = _emit(nc)
    return _NC


def _bf(a):
    return np.ascontiguousarray(np.asarray(a, dtype=np.float32).astype(ml_dtypes.bfloat16))


def build_in_maps(inputs):
    x = np.asarray(inputs["x_input"], dtype=np.float32)
    W1 = _bf(inputs["W1"])
    Wx = _bf(inputs["Wx"])
    W2 = _bf(inputs["W2"])
    b1 = np.ascontiguousarray(np.asarray(inputs["b1"], dtype=np.float32))
    b2 = np.ascontiguousarray(np.asarray(inputs["b2"], dtype=np.float32))
    in_maps = []
    for c in range(NCORES):
        b, s0 = c // 4, (c % 4) * RPC
        gmask = np.zeros((1, NCORES), np.float32)
        gmask[0, 4 * b:4 * b + 4] = 1.0
        in_maps.append({
            "xT": _bf(x[b, s0:s0 + RPC, :].T),
            "W1": W1, "Wx": Wx, "W2": W2, "b1": b1, "b2": b2,
            "gmask": gmask,
        })
    return in_maps


def kernel(**inputs):
    nc = _get_nc()
    in_maps = build_in_maps(inputs)
    res = run_bass_kernel_spmd(nc, in_maps, core_ids=list(range(NCORES)))
    out = np.zeros((B, S, D), np.float32)
    for c, om in enumerate(res.results):
        b, s0 = c // 4, (c % 4) * RPC
        out[b, s0:s0 + RPC, :] = om["zT_out"].T
    return out


# revision 77
# speedup vs baseline: 1.1523x; 1.1523x over previous
"""Trainium2 Bass kernel for the DeepEquilibriumModel (Anderson-accelerated DEQ).

Problem: 12 unrolled iterations of
    f(z) = tanh(z @ W1 + x @ Wx + b1) @ W2 + b2
with Anderson mixing (M=5, beta=1, lam=1e-4) from iteration 5 on.

Sharding: data parallel over the 2048 = B*S rows; 8 cores get 256 rows each
(cores 0-3 hold batch 0, cores 4-7 batch 1). Weights replicated. The Anderson
normal equations need global row sums per batch element -> tiny per-group
AllReduce ([1,20] floats, groups {0..3} / {4..7}).

Everything on-chip is transposed ([feature, row]); bf16 for all big tensors
(validated: 0.4% noise on f per iteration -> ~5e-3 final rel err, gate 2e-2).

Key restructure vs a straightforward port (z-space GEMM1 each iter):
  u_j := W1.T @ f_j + (Wx.T @ x + b1)      (computed right after f_j, i.e.
                                            DURING the AllReduce+solve window)
  z_{i+1} = sum_k c_k f_{i-k}   (coeffs c sum to 1)
  =>  pre-tanh input v_{i+1} = sum_k c_k u_{i-k}    (scaled-identity matmuls
                                                     on the PE, not DVE passes)
so the PE never idles long enough for the HAM clock gate to re-throttle, and
the serial AllReduce/solve chain overlaps the u-GEMM. GEMM2 runs m-chunk-major
so g and the Gram dots pipeline chunk-wise behind it.
"""

import os as _os

import numpy as np
import ml_dtypes

from concourse import bacc, bass, mybir, tile
from concourse.bass_utils import run_bass_kernel_spmd

B, S, D, F = 2, 1024, 512, 2048
MAX_ITER, M, LAM = int(_os.environ.get("K_ITERS", "12")), 5, 1e-4
NCORES = 8
RPC = (B * S) // NCORES      # rows per core = 256
KD = D // 128                # 4 chunks over D
KF = F // 128                # 16 chunks over F
MD = D // 128                # 4 output chunks over D

FP = mybir.dt.float32
BF = mybir.dt.bfloat16
ALU = mybir.AluOpType
ACT = mybir.ActivationFunctionType

RGROUPS = [[0, 1, 2, 3], [4, 5, 6, 7]]

N_DUMMY = int(_os.environ.get("K_DUMMY", "28"))      # keep-HAM-warm matmuls
N_WARM = int(_os.environ.get("K_CC_WARMUP", "2"))    # collective path warmups
USE_AG = _os.environ.get("K_USE_AG", "1") == "1"     # AllGather + local sum
USE_A2A = _os.environ.get("K_USE_A2A", "1") == "1"   # mesh AllToAll as AllGather
NDOT = 20                                            # 5 dots x 4 m-chunk slots
NEDOT = 84                                           # batched early dots
EB = int(_os.environ.get("K_EBATCH", "4"))           # iters 0..EB in one batched AllReduce


def _emit(nc: bass.Bass):
    v = nc.vector
    sc = nc.scalar
    gp = nc.gpsimd

    xT_d = nc.dram_tensor("xT", [D, RPC], BF, kind="ExternalInput")
    W1_d = nc.dram_tensor("W1", [D, F], BF, kind="ExternalInput")
    Wx_d = nc.dram_tensor("Wx", [D, F], BF, kind="ExternalInput")
    W2_d = nc.dram_tensor("W2", [F, D], BF, kind="ExternalInput")
    b1_d = nc.dram_tensor("b1", [F], FP, kind="ExternalInput")
    b2_d = nc.dram_tensor("b2", [D], FP, kind="ExternalInput")
    gm_d = nc.dram_tensor("gmask", [1, NCORES], FP, kind="ExternalInput")
    zout_d = nc.dram_tensor("zT_out", [D, RPC], FP, kind="ExternalOutput")

    with tile.TileContext(nc) as tc:
        with (
            tc.tile_pool(name="const", bufs=1) as cp,
            tc.tile_pool(name="state", bufs=1) as sp,
            tc.tile_pool(name="psv", bufs=3, space="PSUM") as ppv,
            tc.tile_pool(name="psu", bufs=2, space="PSUM") as ppu,
            tc.tile_pool(name="ps2", bufs=2, space="PSUM") as pp2,
            tc.tile_pool(name="pssm", bufs=1, space="PSUM") as pps,
            tc.tile_pool(name="dram", bufs=2, space="DRAM") as dp,
        ):
            # ---------------- constants / weights ----------------
            W1p = cp.tile([128, KD * F], BF)          # (k,f) at [:, k*F + f*128]
            W2p = cp.tile([128, KF * D], BF)          # (f,m) at [:, f*D + m*128]
            Wxp = cp.tile([128, KD * F], BF)
            xTs = cp.tile([128, KD * RPC], BF)        # k at [:, k*RPC]
            xwxb = cp.tile([128, KF * RPC], BF)       # Wx.T x + b1, f at [:, f*RPC]
            b1t = cp.tile([128, KF], FP)
            b2t = cp.tile([128, MD], FP)
            ones_col = cp.tile([128, 1], FP)
            ones_row = cp.tile([1, 128], FP)
            onesq = cp.tile([128, 128], FP)
            identB = cp.tile([128, 128], BF)

            # load order = consumption order: x+Wx+b1 (xwx GEMM), W2 (GEMM2),
            # W1 (u-GEMM) last
            nc.sync.dma_start(b1t[:], b1_d.ap().rearrange("(f p) -> p f", p=128))
            nc.sync.dma_start(b2t[:], b2_d.ap().rearrange("(m p) -> p m", p=128))
            for k in range(KD):
                nc.sync.dma_start(xTs[:, k * RPC:(k + 1) * RPC], xT_d[k * 128:(k + 1) * 128, :])
            for k in range(KD):
                nc.sync.dma_start(Wxp[:, k * F:(k + 1) * F], Wx_d[k * 128:(k + 1) * 128, :])
            for f in range(KF):
                nc.sync.dma_start(W2p[:, f * D:(f + 1) * D], W2_d[f * 128:(f + 1) * 128, :])
            for k in range(KD):
                nc.sync.dma_start(W1p[:, k * F:(k + 1) * F], W1_d[k * 128:(k + 1) * 128, :])
            v.memset(ones_col[:], 1.0)
            v.memset(ones_row[:], 1.0)
            v.memset(onesq[:], 1.0)
            gp.affine_select(onesq[:], onesq[:], [[1, 128]], ALU.is_equal, 0.0,
                             base=0, channel_multiplier=-1)
            v.tensor_copy(identB[:], onesq[:])

            # ---------------- persistent state ----------------
            uh = [sp.tile([128, KF * RPC], BF, name=f"uh{j}") for j in range(M)]
            fh = [sp.tile([128, KD * RPC], BF, name=f"fh{j}") for j in range(M)]
            gh = [sp.tile([128, KD * RPC], BF, name=f"gh{j}") for j in range(M)]
            hT = sp.tile([128, KF * RPC], BF)
            zc = sp.tile([128, KD * RPC], BF)         # Anderson-combined z
            zstage = sp.tile([128, KD * RPC], FP)
            zt0 = sp.tile([128, KD * RPC], BF)
            zt1 = sp.tile([128, KD * RPC], BF)
            junk_dve = sp.tile([128, RPC], BF)
            junk_act = sp.tile([128, RPC], BF)
            dots_act = sp.tile([128, MD], FP)         # <g,g> per m-chunk
            dots_dve = sp.tile([128, 2 * MD], FP)     # <g,g_{i-j}> j=1,2 per chunk
            dots_gp = sp.tile([128, 2 * MD], FP)      # <g,g_{i-j}> j=3,4 per chunk
            # batched early (iters 0-4) dot accumulators: one AllReduce total
            edots_act = sp.tile([128, 5 * MD], FP)       # sq: iter i at [:, i*4+c]
            edots_dve = sp.tile([128, 4 * 4 * MD], FP)   # (i-1,j-1,c) i,j=1..4
            redp = sp.tile([1, NEDOT], FP)
            red2 = sp.tile([1, NDOT], FP)
            red2E = sp.tile([1, NEDOT], FP)
            red4g = sp.tile([1, 4 * NDOT], FP)
            red8g = sp.tile([1, NCORES * NDOT], FP)
            gmask = sp.tile([1, NCORES], FP)
            red8p = sp.tile([1, NCORES * NDOT], FP)
            red5 = sp.tile([1, M], FP)
            Pg = [sp.tile([1, 25], FP, name=f"pg{j}") for j in range(2)]
            cIb = sp.tile([128, M * 128], BF)         # scaled identities (bf16)
            aug = sp.tile([1, 20], FP)                # [HTH | HTy] augmented 4x5
            outer = sp.tile([1, 20], FP)
            t45 = sp.tile([1, 20], FP)
            rct = sp.tile([1, 4], FP)
            gam = sp.tile([1, 4], FP)
            csum = sp.tile([1, 1], FP)
            coeffs = sp.tile([1, M], FP)
            gst = sp.tile([1, 8], FP)

            nc.sync.dma_start(gmask[:], gm_d[:, :])
            v.memset(red8p[:], 0.0)
            v.memset(dots_act[:], 0.0)
            v.memset(dots_dve[:], 0.0)
            v.memset(dots_gp[:], 0.0)
            v.memset(edots_act[:], 0.0)
            v.memset(edots_dve[:], 0.0)
            v.memset(redp[:], 0.0)
            v.memset(Pg[0][:], 0.0)
            v.memset(Pg[1][:], 0.0)

            # warm the collective paths (first op of a kind pays big latency)
            wcc_in = dp.tile([1, NEDOT], FP, tag="ecci", name="wcci")
            wcc_out = dp.tile([1, NEDOT], FP, tag="ecco", name="wcco")
            gp.dma_start(wcc_in[:], redp[:])
            gp.collective_compute(
                "AllReduce", ALU.add, replica_groups=RGROUPS,
                ins=[wcc_in.opt()], outs=[wcc_out.opt()],
            )
            for w in range(N_WARM):
                if USE_A2A:
                    wag_in = dp.tile([1, NCORES * NDOT], FP, tag="cci", name="wagi")
                    wag_out = dp.tile([1, NCORES * NDOT], FP, tag="cco", name="wago")
                    gp.dma_start(wag_in[:], red8p[:])
                    gp.collective_compute(
                        "AllToAll", ALU.bypass,
                        replica_groups=[list(range(NCORES))],
                        ins=[wag_in.opt()], outs=[wag_out.opt()],
                    )
                elif USE_AG:
                    wag_in = dp.tile([1, NDOT], FP, tag="cci", name="wagi")
                    wag_out = dp.tile([1, 4 * NDOT], FP, tag="cco", name="wago")
                    gp.dma_start(wag_in[:], redp[:, 0:NDOT])
                    gp.collective_compute(
                        "AllGather", ALU.bypass, replica_groups=RGROUPS,
                        ins=[wag_in.opt()], outs=[wag_out.opt()],
                    )
                else:
                    wag_in = dp.tile([1, NDOT], FP, tag="cci", name="wagi")
                    wag_out = dp.tile([1, NDOT], FP, tag="cco", name="wago")
                    gp.dma_start(wag_in[:], redp[:, 0:NDOT])
                    gp.collective_compute(
                        "AllReduce", ALU.add, replica_groups=RGROUPS,
                        ins=[wag_in.opt()], outs=[wag_out.opt()],
                    )

            # ---------------- xwxb = Wx.T @ xT + b1 ; h0 = tanh(xwxb) ----------
            for f in range(KF):
                ps = ppv.tile([128, RPC], FP, tag="psv", name="psx")
                for k in range(KD):
                    nc.tensor.matmul(
                        ps[:],
                        Wxp[:, k * F + f * 128: k * F + (f + 1) * 128],
                        xTs[:, k * RPC:(k + 1) * RPC],
                        start=(k == 0), stop=(k == KD - 1),
                    )
                sc.activation(xwxb[:, f * RPC:(f + 1) * RPC], ps[:],
                              ACT.Identity, bias=b1t[:, f:f + 1], scale=1.0)
                sc.activation(hT[:, f * RPC:(f + 1) * RPC], ps[:],
                              ACT.Tanh, bias=b1t[:, f:f + 1], scale=1.0)

            # ---------------- main loop (fully unrolled) ----------------
            # z_ap: AP of z_i (for g); None at i=0 (z=0)
            z_ap = None

            for i in range(MAX_ITER):
                slot = i % M
                f_t, g_t = fh[slot], gh[slot]

                # ---- pre-tanh input + tanh -> hT (i>=1; i==0 done above) ----
                if 1 <= i <= M:
                    # z_i = f_{i-1}: v_i = u_{i-1} directly
                    up = uh[(i - 1) % M]
                    for f in range(KF):
                        sc.activation(hT[:, f * RPC:(f + 1) * RPC],
                                      up[:, f * RPC:(f + 1) * RPC], ACT.Tanh)
                elif i > M:
                    # v_i = sum_k c_k u_{i-1-k}   (scaled-identity matmuls)
                    for f in range(KF):
                        ps = ppv.tile([128, RPC], FP, tag="psv", name="psv")
                        for k in range(M):
                            nc.tensor.matmul(
                                ps[:],
                                cIb[:, k * 128:(k + 1) * 128],
                                uh[(i - 1 - k) % M][:, f * RPC:(f + 1) * RPC],
                                start=(k == 0), stop=(k == M - 1),
                            )
                        sc.activation(hT[:, f * RPC:(f + 1) * RPC], ps[:], ACT.Tanh)

                # ---- P history bookkeeping ----
                Pc, Pp = Pg[i % 2], Pg[(i + 1) % 2]
                P3c = Pc[:].rearrange("p (a b) -> p a b", a=5)
                P3p = Pp[:].rearrange("p (a b) -> p a b", a=5)
                if i == EB + 1:
                    # replay iters 0-3 from the single batched AllReduce, on
                    # the otherwise-idle GPSIMD engine (overlaps this
                    # iteration's GEMMs on PE / z ops on DVE)
                    def gp_fold4(dst, src):
                        # dst [1,n] = per-group-of-4 sums of src [1,4n]
                        n = dst.shape[-1]
                        s3 = src.rearrange("p (j c) -> p j c", j=n)
                        g3 = gst[:, 0:2 * n].rearrange("p (j c) -> p j c", j=n)
                        gp.tensor_tensor(g3, s3[:, :, 0:2], s3[:, :, 2:4],
                                         op=ALU.add)
                        gp.tensor_tensor(dst, g3[:, :, 0:1].rearrange(
                            "p j c -> p (j c)"), g3[:, :, 1:2].rearrange(
                            "p j c -> p (j c)"), op=ALU.add)

                    for ii in range(EB + 1):
                        Pcc, Ppp = Pg[ii % 2], Pg[(ii + 1) % 2]
                        P3cc = Pcc[:].rearrange("p (a b) -> p a b", a=5)
                        P3pp = Ppp[:].rearrange("p (a b) -> p a b", a=5)
                        if ii > 0:
                            gp.tensor_copy(P3cc[:, 1:5, 1:5], P3pp[:, 0:4, 0:4])
                        gp_fold4(red5[:, 0:1], red2E[:, ii * 4:(ii + 1) * 4])
                        if ii > 0:
                            base = NDOT + (ii - 1) * 16
                            gp_fold4(red5[:, 1:5], red2E[:, base:base + 16])
                        else:
                            gp.memset(red5[:, 1:5], 0.0)
                        gp.tensor_copy(Pcc[:, 0:5], red5[:, 0:5])
                        gp.tensor_copy(Pcc[:, 5:25:5], red5[:, 1:5])
                if i >= EB + 1:
                    # shift by one age (uses only last iter's P; runs early)
                    v.tensor_copy(P3c[:, 1:5, 1:5], P3p[:, 0:4, 0:4])
                if i >= M:
                    # precompute the old-pairs part of HTH (+ridge) so the
                    # post-AllReduce solve prep is shorter
                    T3 = t45[:].rearrange("p (a b) -> p a b", a=4)[:, :, 0:4]
                    v.tensor_copy(T3, P3c[:, 1:5, 1:5])
                    v.tensor_scalar(t45[:, 0:19:6], t45[:, 0:19:6], LAM, None,
                                    op0=ALU.add)

                # ---- GEMM2 m-chunk-major; g + dots pipeline behind it ----
                njd = min(i, M - 1)
                for m in range(MD):
                    ps2 = pp2.tile([128, RPC], FP, tag="ps2", name="ps2")
                    for f in range(KF):
                        nc.tensor.matmul(
                            ps2[:],
                            W2p[:, f * D + m * 128: f * D + (m + 1) * 128],
                            hT[:, f * RPC:(f + 1) * RPC],
                            start=(f == 0), stop=(f == KF - 1),
                        )
                    sl = slice(m * RPC, (m + 1) * RPC)
                    sc.activation(f_t[:, sl], ps2[:], ACT.Identity,
                                  bias=b2t[:, m:m + 1], scale=1.0)
                    # g straight from PSUM (doesn't wait on the bias ACT)
                    if i == 0:
                        v.tensor_scalar(g_t[:, sl], ps2[:], b2t[:, m:m + 1], None,
                                        op0=ALU.add)
                    else:
                        v.scalar_tensor_tensor(g_t[:, sl], ps2[:], b2t[:, m:m + 1],
                                               z_ap[:, sl],
                                               op0=ALU.add, op1=ALU.subtract)
                    da = (dots_act[:, m:m + 1] if i > EB
                          else edots_act[:, i * MD + m:i * MD + m + 1])
                    sc.activation(junk_act[:], g_t[:, sl], ACT.Square,
                                  accum_out=da)
                    for j in range(1, njd + 1):
                        if i > EB:
                            if j <= 2:
                                dd = dots_dve[:, (j - 1) * MD + m:(j - 1) * MD + m + 1]
                            else:
                                dd = dots_gp[:, (j - 3) * MD + m:(j - 3) * MD + m + 1]
                        else:
                            c0 = ((i - 1) * 4 + (j - 1)) * MD + m
                            dd = edots_dve[:, c0:c0 + 1]
                        v.scalar_tensor_tensor(
                            junk_dve[:], g_t[:, sl], 1.0,
                            gh[(i - j) % M][:, sl],
                            op0=ALU.bypass, op1=ALU.mult,
                            accum_out=dd,
                        )

                # ---- partition-reduce dots, ship through AllReduce ----
                pball = pps.tile([128, 96], FP, tag="psmall", name="pball")
                if i == EB:
                    # one batched AllReduce for the whole early history
                    nc.tensor.matmul(pball[0:1, 0:NDOT], ones_col[:],
                                     edots_act[:], start=True, stop=True)
                    nc.tensor.matmul(pball[0:1, NDOT:NEDOT], ones_col[:],
                                     edots_dve[:], start=True, stop=True)
                    sc.activation(redp[:], pball[0:1, 0:NEDOT], ACT.Copy)
                    ecc_in = dp.tile([1, NEDOT], FP, tag="ecci", name="ecci")
                    ecc_out = dp.tile([1, NEDOT], FP, tag="ecco", name="ecco")
                    nc.sync.dma_start(ecc_in[:], redp[:])
                    gp.collective_compute(
                        "AllReduce", ALU.add, replica_groups=RGROUPS,
                        ins=[ecc_in.opt()], outs=[ecc_out.opt()],
                    )
                    nc.sync.dma_start(red2E[:], ecc_out[:])
                elif i > EB:
                    nc.tensor.matmul(pball[0:1, 0:MD], ones_col[:], dots_act[:],
                                     start=True, stop=True)
                    nc.tensor.matmul(pball[0:1, MD:3 * MD], ones_col[:], dots_dve[:],
                                     start=True, stop=True)
                    nc.tensor.matmul(pball[0:1, 3 * MD:NDOT], ones_col[:], dots_gp[:],
                                     start=True, stop=True)
                    if USE_A2A:
                        # 8-rank mesh AllToAll as a masked AllGather: shard r
                        # carries my dots if rank r is in my batch group, else
                        # zeros -- the fold then sums all 8 shards and the
                        # other group contributes nothing
                        v.tensor_tensor(
                            red8p[:].rearrange("p (r f) -> p r f", r=NCORES),
                            pball[0:1, 0:NDOT].rearrange(
                                "p (r f) -> p r f", r=1).broadcast_to(
                                    [1, NCORES, NDOT]),
                            gmask[:].rearrange("p (r f) -> p r f", f=1
                                               ).broadcast_to([1, NCORES, NDOT]),
                            op=ALU.mult)
                        cc_in = dp.tile([1, NCORES * NDOT], FP, tag="cci", name="cci")
                        nc.sync.dma_start(cc_in[:], red8p[:])
                        cc_out = dp.tile([1, NCORES * NDOT], FP, tag="cco", name="cco")
                        gp.collective_compute(
                            "AllToAll", ALU.bypass,
                            replica_groups=[list(range(NCORES))],
                            ins=[cc_in.opt()], outs=[cc_out.opt()],
                        )
                        nc.sync.dma_start(red8g[:], cc_out[:])
                    elif USE_AG:
                        sc.activation(redp[:, 0:NDOT], pball[0:1, 0:NDOT], ACT.Copy)
                        cc_in = dp.tile([1, NDOT], FP, tag="cci", name="cci")
                        nc.sync.dma_start(cc_in[:], redp[:, 0:NDOT])
                        cc_out = dp.tile([1, 4 * NDOT], FP, tag="cco", name="cco")
                        gp.collective_compute(
                            "AllGather", ALU.bypass, replica_groups=RGROUPS,
                            ins=[cc_in.opt()], outs=[cc_out.opt()],
                        )
                        nc.sync.dma_start(red4g[:], cc_out[:])
                    else:
                        sc.activation(redp[:, 0:NDOT], pball[0:1, 0:NDOT], ACT.Copy)
                        cc_in = dp.tile([1, NDOT], FP, tag="cci", name="cci")
                        cc_out = dp.tile([1, NDOT], FP, tag="cco", name="cco")
                        nc.sync.dma_start(cc_in[:], redp[:, 0:NDOT])
                        gp.collective_compute(
                            "AllReduce", ALU.add, replica_groups=RGROUPS,
                            ins=[cc_in.opt()], outs=[cc_out.opt()],
                        )
                        nc.sync.dma_start(red2[:], cc_out[:])

                # ---- u_i = W1.T @ f_i + xwxb  (runs during the AllReduce).
                # For early iterations the DVE is idle and has no post-
                # collective chain queued, so the xwxb fold rides a DVE
                # epilogue there instead of 16 identity matmuls; Anderson
                # iterations keep the PE fold so the DVE queue stays clear
                # ahead of the solve.
                if i < MAX_ITER - 1:
                    u_t = uh[slot]
                    fold_on_pe = i > EB
                    for f in range(KF):
                        psu = ppu.tile([128, RPC], FP, tag="psu", name="psu")
                        if fold_on_pe:
                            nc.tensor.matmul(psu[:], identB[:],
                                             xwxb[:, f * RPC:(f + 1) * RPC],
                                             start=True, stop=False)
                        for k in range(KD):
                            nc.tensor.matmul(
                                psu[:],
                                W1p[:, k * F + f * 128: k * F + (f + 1) * 128],
                                f_t[:, k * RPC:(k + 1) * RPC],
                                start=(not fold_on_pe and k == 0),
                                stop=(k == KD - 1),
                            )
                        if fold_on_pe:
                            sc.activation(u_t[:, f * RPC:(f + 1) * RPC], psu[:],
                                          ACT.Copy)
                        else:
                            v.scalar_tensor_tensor(
                                u_t[:, f * RPC:(f + 1) * RPC], psu[:], 1.0,
                                xwxb[:, f * RPC:(f + 1) * RPC],
                                op0=ALU.bypass, op1=ALU.add)
                # keep the PE's HAM clock warm through the post-collective
                # solve window (first two share PSUM slots with the tail
                # u-copies -- freed within ~0.5us)
                if i >= M:
                    for dmy in range(N_DUMMY):
                        psd = ppu.tile([128, RPC], FP, tag="psu", name="psu")
                        nc.tensor.matmul(psd[:], identB[:], xwxb[:, 0:RPC],
                                         start=True, stop=True)

                if i <= EB:
                    z_ap = f_t
                    continue

                # ---- fold AllReduced/AllGathered dots, insert into P ----
                if USE_A2A:
                    v.tensor_reduce(
                        red5[:],
                        red8g[:].rearrange("p (r j c) -> p j r c", r=NCORES, j=M),
                        axis=mybir.AxisListType.XY, op=ALU.add)
                elif USE_AG:
                    # one strided-view reduce: [1,(r j c)] -> [1,j,(r c)] -> [1,j]
                    v.tensor_reduce(
                        red5[:],
                        red4g[:].rearrange("p (r j c) -> p j r c", r=4, j=M),
                        axis=mybir.AxisListType.XY, op=ALU.add)
                else:
                    v.tensor_reduce(red5[:],
                                    red2[:].rearrange("p (j c) -> p j c", j=M),
                                    axis=mybir.AxisListType.X, op=ALU.add)
                if i < M:
                    v.tensor_copy(Pc[:, 0:5], red5[:, 0:5])
                    v.tensor_copy(Pc[:, 5:25:5], red5[:, 1:5])
                    z_ap = f_t
                    continue

                # ---- finish [HTH + lam I | HTy] (4x5, in aug) ----
                # HTH[a][b] = P00 - P0b - Pa0 + Pab ; HTy[a] = P00 - Pa0.
                # P is symmetric so the new row/col 0 IS red5 -- read it
                # directly; the P inserts happen after the solve, off the
                # critical path (only next iteration's shift needs them).
                A3 = aug[:].rearrange("p (a b) -> p a b", a=4)
                H3 = A3[:, :, 0:4]
                T3 = t45[:].rearrange("p (a b) -> p a b", a=4)[:, :, 0:4]
                r5r = red5[:, 1:5].rearrange("p (a b) -> p a b", a=1
                                             ).broadcast_to([1, 4, 4])
                r5c = red5[:, 1:5].rearrange("p (a b) -> p a b", b=1
                                             ).broadcast_to([1, 4, 4])
                v.scalar_tensor_tensor(T3, r5r, -1.0, T3,
                                       op0=ALU.mult, op1=ALU.add)
                v.tensor_tensor(T3, T3, r5c, op=ALU.subtract)
                v.tensor_scalar(H3, T3, red5[:, 0:1], None, op0=ALU.add)
                v.scalar_tensor_tensor(A3[:, :, 4:5],
                                       red5[:, 1:5].rearrange(
                                           "p (a b) -> p a b", b=1),
                                       -1.0,
                                       red5[:, 0:1].broadcast_to([1, 4]).rearrange(
                                           "p (a b) -> p a b", b=1),
                                       op0=ALU.mult, op1=ALU.add)

                # ---- Gauss-Jordan on [A|y]: gamma lands in aug col 4 ----
                O3 = outer[:].rearrange("p (a b) -> p a b", a=4)
                for p in range(4):
                    rp = rct[:, p:p + 1]
                    v.reciprocal(rp, aug[:, p * 5 + p:p * 5 + p + 1])
                    v.scalar_tensor_tensor(
                        O3, A3[:, :, p:p + 1].broadcast_to([1, 4, 5]), rp,
                        A3[:, p:p + 1, :].broadcast_to([1, 4, 5]),
                        op0=ALU.mult, op1=ALU.mult)
                    v.tensor_tensor(A3, A3, O3, op=ALU.subtract)
                    v.tensor_scalar(aug[:, p * 5:(p + 1) * 5],
                                    outer[:, p * 5:(p + 1) * 5], rp, None,
                                    op0=ALU.mult)

                # ---- coeffs = [1 - sum(gamma), gamma] -> broadcast + cI ----
                v.tensor_copy(gam[:], aug[:, 4:20:5])
                v.tensor_reduce(csum[:], gam[:], axis=mybir.AxisListType.X, op=ALU.add)
                v.tensor_scalar(coeffs[:, 0:1], csum[:], -1.0, 1.0,
                                op0=ALU.mult, op1=ALU.add)
                v.tensor_copy(coeffs[:, 1:5], gam[:])
                psb = pball[:, 24:29]
                nc.tensor.matmul(psb, ones_row[:], coeffs[:], start=True, stop=True)
                for k in range(M):
                    v.tensor_scalar(cIb[:, k * 128:(k + 1) * 128],
                                    identB[:], psb[:, k:k + 1], None, op0=ALU.mult)
                # deferred P inserts (next iteration's shift needs them)
                v.tensor_copy(Pc[:, 0:5], red5[:, 0:5])
                v.tensor_copy(Pc[:, 5:25:5], red5[:, 1:5])

                # ---- z_{i+1} = sum_k c_k f_{i-k} on DVE (hidden under next
                #      iteration's PE phase); full-width ops to cut per-op
                #      overhead; final iteration uses PE instead ----
                if i < MAX_ITER - 1:
                    v.tensor_scalar(zt0[:], fh[i % M][:], psb[:, 0:1], None,
                                    op0=ALU.mult)
                    cur = zt0
                    for k in range(1, M):
                        dst = zc if k == M - 1 else (zt1 if cur is zt0 else zt0)
                        v.scalar_tensor_tensor(dst[:], fh[(i - k) % M][:],
                                               psb[:, k:k + 1], cur[:],
                                               op0=ALU.mult, op1=ALU.add)
                        cur = dst
                    z_ap = zc

            # ---------------- final z = sum_k c_k f_{11-k} on PE ----------------
            li = MAX_ITER - 1
            for m in range(MD):
                ps = ppv.tile([128, RPC], FP, tag="psv", name="psf")
                for k in range(M):
                    nc.tensor.matmul(
                        ps[:], cIb[:, k * 128:(k + 1) * 128],
                        fh[(li - k) % M][:, m * RPC:(m + 1) * RPC],
                        start=(k == 0), stop=(k == M - 1),
                    )
                sc.activation(zstage[:, m * RPC:(m + 1) * RPC], ps[:], ACT.Copy)
            for k in range(KD):
                nc.sync.dma_start(zout_d[k * 128:(k + 1) * 128, :],
                                  zstage[:, k * RPC:(k + 1) * RPC])

    nc.compile()
    nc.finalize()
    return nc


_NC = None


def _get_nc():
    global _NC
    if _NC is None:
        nc = bacc.Bacc(trn_type="TRN2", debug=False, num_devices=NCORES)
        _NC = _emit(nc)
    return _NC


def _bf(a):
    return np.ascontiguousarray(np.asarray(a, dtype=np.float32).astype(ml_dtypes.bfloat16))


def build_in_maps(inputs):
    x = np.asarray(inputs["x_input"], dtype=np.float32)
    W1 = _bf(inputs["W1"])
    Wx = _bf(inputs["Wx"])
    W2 = _bf(inputs["W2"])
    b1 = np.ascontiguousarray(np.asarray(inputs["b1"], dtype=np.float32))
    b2 = np.ascontiguousarray(np.asarray(inputs["b2"], dtype=np.float32))
    in_maps = []
    for c in range(NCORES):
        b, s0 = c // 4, (c % 4) * RPC
        gmask = np.zeros((1, NCORES), np.float32)
        gmask[0, 4 * b:4 * b + 4] = 1.0
        in_maps.append({
            "xT": _bf(x[b, s0:s0 + RPC, :].T),
            "W1": W1, "Wx": Wx, "W2": W2, "b1": b1, "b2": b2,
            "gmask": gmask,
        })
    return in_maps


def kernel(**inputs):
    nc = _get_nc()
    in_maps = build_in_maps(inputs)
    res = run_bass_kernel_spmd(nc, in_maps, core_ids=list(range(NCORES)))
    out = np.zeros((B, S, D), np.float32)
    for c, om in enumerate(res.results):
        b, s0 = c // 4, (c % 4) * RPC
        out[b, s0:s0 + RPC, :] = om["zT_out"].T
    return out
